# revision 1
# baseline (speedup 1.0000x reference)
"""Multi-head causal attention (LLaMA-style RoPE) on 8 Trainium2 NeuronCores.

Sharding: data-parallel, zero-communication. Core c handles batch c//2 and
query rows [512*(c%2), 512*(c%2)+512). Each core computes K/V projections for
all 1024 rows of its batch (duplicated across the core pair), its own Q half,
attention, and the output projection for its rows. The compiled program is
identical on all cores; per-core differences live only in the input data
(x slices, rotary columns, additive score mask).

Numerics: all matmuls run as float32r (full-rate fp32 on the PE systolic
array, ~1.5e-4 rms vs fp32). Scores are computed transposed (ST[k,q]) so the
probability matrix lands directly in the layout the PV matmul needs — no
transposes anywhere. Softmax skips max-subtraction (logits are O(5) here) and
gets the denominator from a ones-matrix matmul accumulated alongside PV.
The additive mask input is applied to raw scores before exp, so any mask
content (not just causal) is honored.
"""

import math
import sys

import numpy as np

sys.path.insert(0, "/opt/trn_rl_repo")

B, S, DIM, H = 4, 1024, 2048, 16
HD = DIM // H  # 128
NQ = S // 2  # query rows per core
KC = DIM // 128  # contraction chunks for projections
NKT = S // 128  # key tiles
SCALE = 1.0 / math.sqrt(HD)
N_CORES = 8

_cache = {}


def _build_nc():
    import concourse.mybir as mybir
    import concourse.tile as tile
    from concourse import bacc

    F32R = mybir.dt.float32r
    F32 = mybir.dt.float32

    nc = bacc.Bacc("TRN2", target_bir_lowering=False, debug=False,
                   num_devices=N_CORES)

    x_in = nc.dram_tensor("x_pre", [128, KC, S], F32R, kind="ExternalInput")
    wq_in = nc.dram_tensor("wq_pre", [H, 128, KC, 128], F32R, kind="ExternalInput")
    wk_in = nc.dram_tensor("wk_pre", [H, 128, KC, 128], F32R, kind="ExternalInput")
    wv_in = nc.dram_tensor("wv_pre", [4, 4, 128, 4, 512], F32R, kind="ExternalInput")
    wo_in = nc.dram_tensor("wo_pre", [4, 4, 128, 4, 512], F32R, kind="ExternalInput")
    bq_in = nc.dram_tensor("bq_p", [128, KC, 1], F32, kind="ExternalInput")
    bk_in = nc.dram_tensor("bk_p", [128, KC, 1], F32, kind="ExternalInput")
    bv_in = nc.dram_tensor("bv128", [128, DIM], F32, kind="ExternalInput")
    csk_in = nc.dram_tensor("csk2", [128, S], F32R, kind="ExternalInput")
    ssk_in = nc.dram_tensor("ssk2", [128, S], F32R, kind="ExternalInput")
    amask_in = nc.dram_tensor("amask", [128, NKT, NQ], F32R, kind="ExternalInput")
    ident_in = nc.dram_tensor("ident128", [128, 128], F32R, kind="ExternalInput")
    ones_in = nc.dram_tensor("ones128", [128, 128], F32R, kind="ExternalInput")
    y_out = nc.dram_tensor("y", [NQ, DIM], F32, kind="ExternalOutput")

    with tile.TileContext(nc) as tc:
        with (
            tc.tile_pool(name="consts", bufs=1) as consts,
            tc.tile_pool(name="xpool", bufs=1) as xpool,
            tc.tile_pool(name="vres", bufs=1) as vres,
            tc.tile_pool(name="trig", bufs=2) as trig,
            tc.tile_pool(name="maskpool", bufs=1) as maskpool,
            tc.tile_pool(name="wpool", bufs=3) as wpool,
            tc.tile_pool(name="rope", bufs=2) as rope,
            tc.tile_pool(name="attn", bufs=3) as attn,
            tc.tile_pool(name="dram", bufs=1, space="DRAM") as dram,
            tc.tile_pool(name="ps_mm", bufs=3, space="PSUM") as ps_mm,
            tc.tile_pool(name="ps_st", bufs=3, space="PSUM") as ps_st,
            tc.tile_pool(name="ps_l", bufs=1, space="PSUM") as ps_l,
            tc.tile_pool(name="ps_o", bufs=1, space="PSUM") as ps_o,
        ):
            qt_d = dram.tile([H, 128, NQ], F32R)
            kt_d = dram.tile([H, 128, S], F32R)

            ones_sb = consts.tile([128, 128], F32R)
            nc.sync.dma_start(ones_sb[:], ones_in[:])
            ident_sb = consts.tile([128, 128], F32R)
            nc.sync.dma_start(ident_sb[:], ident_in[:])
            bq_sb = consts.tile([128, KC, 1], F32, tag="bq")
            bk_sb = consts.tile([128, KC, 1], F32, tag="bk")
            nc.sync.dma_start(bq_sb[:], bq_in[:])
            nc.sync.dma_start(bk_sb[:], bk_in[:])

            def rope_block(psum, b_sb, cs_sb, ss_sb, n, dst_ap):
                """dst = rope(psum + bias); all [128, n] partition-aligned."""
                tmp_t = rope.tile([128, 512], F32R, tag="tmp", name="tmp_t")
                tmp = tmp_t[:, :n]
                nc.scalar.activation(
                    tmp, psum, mybir.ActivationFunctionType.Identity, bias=b_sb
                )
                tswap_t = rope.tile([128, 512], F32R, tag="tswap", name="tswap_t")
                tswap = tswap_t[:, :n]
                nc.sync.dma_start(tswap[0:64, :], tmp[64:128, :])
                nc.sync.dma_start(tswap[64:128, :], tmp[0:64, :])
                nc.vector.tensor_mul(tmp, tmp, cs_sb)
                nc.vector.tensor_mul(tswap, tswap, ss_sb)
                nc.vector.tensor_add(dst_ap, tmp, tswap)

            # ---------------- Q+K projections + RoPE (head-interleaved) ----------------
            # x columns are host-rotated: [own q-half rows, other-half rows],
            # so Q projection is uniformly the first NQ columns.
            csk_sb = trig.tile([128, S], F32R, tag="trig", name="csk_sb")
            ssk_sb = trig.tile([128, S], F32R, tag="trig", name="ssk_sb")
            nc.sync.dma_start(csk_sb[:], csk_in[:])
            nc.sync.dma_start(ssk_sb[:], ssk_in[:])
            x_sb = xpool.tile([128, KC, S], F32R, tag="x", name="x_sb")
            for kq in range(4):
                nc.sync.dma_start(x_sb[:, kq * 4:kq * 4 + 4, :],
                                  x_in[:, kq * 4:kq * 4 + 4, :])
            for h in range(H):
                w_sb = wpool.tile([128, KC, 128], F32R, tag="w")
                nc.sync.dma_start(w_sb[:], wq_in[h])
                pm_t = ps_mm.tile([128, 512], F32, tag="mm", name="pm_t")
                pm = pm_t[:, :NQ]
                for kc in range(KC):
                    nc.tensor.matmul(pm, w_sb[:, kc, :], x_sb[:, kc, :NQ],
                                     start=(kc == 0), stop=(kc == KC - 1))
                qdst_t = rope.tile([128, 512], F32R, tag="dst", name="qdst_t")
                qdst = qdst_t[:, :NQ]
                rope_block(pm, bq_sb[:, h, :], csk_sb[:, :NQ], ssk_sb[:, :NQ],
                           NQ, qdst)
                nc.sync.dma_start(qt_d[h], qdst)
                wk_sb = wpool.tile([128, KC, 128], F32R, tag="w")
                nc.sync.dma_start(wk_sb[:], wk_in[h])
                for nb in range(2):
                    cols = slice(nb * 512, nb * 512 + 512)
                    pm = ps_mm.tile([128, 512], F32, tag="mm")
                    for kc in range(KC):
                        nc.tensor.matmul(pm[:], wk_sb[:, kc, :], x_sb[:, kc, cols],
                                         start=(kc == 0), stop=(kc == KC - 1))
                    kdst = rope.tile([128, 512], F32R, tag="dst")
                    rope_block(pm[:], bk_sb[:, h, :], csk_sb[:, cols],
                               ssk_sb[:, cols], 512, kdst[:])
                    nc.sync.dma_start(kt_d[h][:, cols], kdst[:])
            # ---------------- V projection -> resident V ----------------
            bv_full = maskpool.tile([128, 4, NQ], F32, tag="bv", name="bv_full")
            nc.sync.dma_start(bv_full[:], bv_in.rearrange("p (a b) -> p a b", a=4))
            v_sb = vres.tile([128, NKT, DIM], F32R)  # [k-within-tile, ktile, d]
            for eb in range(4):
                ecols = slice(eb * 512, eb * 512 + 512)
                vps = [ps_mm.tile([128, 512], F32, tag="mm", name="vps0"),
                       ps_mm.tile([128, 512], F32, tag="mm", name="vps1"),
                       ps_mm.tile([128, 512], F32, tag="mm", name="vps2"),
                       ps_st.tile([128, 512], F32, tag="st", name="vps3"),
                       ps_st.tile([128, 512], F32, tag="st", name="vps4"),
                       ps_st.tile([128, 512], F32, tag="st", name="vps5"),
                       ps_l.tile([128, 512], F32, tag="l", name="vps6"),
                       ps_o.tile([128, 512], F32, tag="o", name="vps7")]
                for kch in range(4):
                    wch = wpool.tile([128, 4, 512], F32R, tag="w", name="wch")
                    nc.sync.dma_start(wch[:], wv_in[kch, eb])
                    for st in range(NKT):
                        scols = slice(st * 128, st * 128 + 128)
                        for dc in range(4):
                            kc = kch * 4 + dc
                            nc.tensor.matmul(vps[st][:], x_sb[:, kc, scols],
                                             wch[:, dc, :],
                                             start=(kc == 0), stop=(kc == KC - 1))
                for st in range(NKT):
                    nc.vector.tensor_add(v_sb[:, st, ecols], vps[st][:],
                                         bv_full[:, eb, :])
            # ---------------- attention per head -> resident OT ----------------
            amask_sb = maskpool.tile([128, NKT, NQ], F32R, tag="mb", name="amask_sb")
            nc.sync.dma_start(amask_sb[:], amask_in[:])
            ot_full = xpool.tile([128, KC, S], F32R, tag="x", name="ot_full")
            ot_sb = ot_full[:, :, :NQ]  # [128, H, NQ]
            for h in range(H):
                qh = attn.tile([128, NQ], F32R, tag="qh")
                nc.sync.dma_start(qh[:], qt_d[h])
                l_ps = ps_l.tile([128, NQ], F32, tag="l")
                o_ps = ps_o.tile([128, NQ], F32, tag="o")
                for kt in range(NKT):
                    kcols = slice(kt * 128, kt * 128 + 128)
                    # rotated k-order makes tiles 0-3 uniformly triangular:
                    # q < kt*128 is invalid on every core, skip it.
                    qv = slice(kt * 128 if kt < 4 else 0, NQ)
                    kh_t = attn.tile([128, 128], F32R, tag="kh")
                    nc.sync.dma_start(kh_t[:], kt_d[h][:, kcols])
                    st_ps = ps_st.tile([128, NQ], F32, tag="st")
                    nc.tensor.matmul(st_ps[:, qv], kh_t[:], qh[:, qv],
                                     start=True, stop=False)
                    # tiles 0-3 need mask only on their diagonal 128 cols
                    # (rest of the restricted range is valid on every core);
                    # tiles 4-7 need it everywhere (all-valid vs all-invalid
                    # cores differ via the mask data).
                    mv = slice(kt * 128, kt * 128 + 128) if kt < 4 else qv
                    nc.tensor.matmul(st_ps[:, mv], ident_sb[:],
                                     amask_sb[:, kt, mv], start=False, stop=True)
                    pt = rope.tile([128, 512], F32R, tag="tswap", name="pt")
                    nc.scalar.activation(pt[:, qv], st_ps[:, qv],
                                         mybir.ActivationFunctionType.Exp,
                                         scale=SCALE)
                    nc.tensor.matmul(l_ps[:, qv], ones_sb[:], pt[:, qv],
                                     start=(kt == 0), stop=(kt == NKT - 1))
                    nc.tensor.matmul(o_ps[:, qv], v_sb[:, kt, h * 128:(h + 1) * 128],
                                     pt[:, qv], start=(kt == 0), stop=(kt == NKT - 1))
                rl = rope.tile([128, 512], F32, tag="tmp", name="rl")[:, :NQ]
                nc.vector.reciprocal_approx_fast(rl[:], l_ps[:])
                nc.vector.tensor_mul(ot_sb[:, h, :], o_ps[:], rl[:])

            # ---------------- output projection ----------------
            for eb in range(4):
                ecols = slice(eb * 512, eb * 512 + 512)
                ops = [ps_mm.tile([128, 512], F32, tag="mm", name="ops0"),
                       ps_mm.tile([128, 512], F32, tag="mm", name="ops1"),
                       ps_st.tile([128, 512], F32, tag="st", name="ops2"),
                       ps_st.tile([128, 512], F32, tag="st", name="ops3")]
                for kch in range(4):
                    wch = wpool.tile([128, 4, 512], F32R, tag="w", name="woch")
                    nc.sync.dma_start(wch[:], wo_in[kch, eb])
                    for st in range(NQ // 128):
                        scols = slice(st * 128, st * 128 + 128)
                        for dc in range(4):
                            dcg = kch * 4 + dc
                            nc.tensor.matmul(ops[st][:], ot_sb[:, dcg, scols],
                                             wch[:, dc, :],
                                             start=(dcg == 0), stop=(dcg == H - 1))
                for st in range(NQ // 128):
                    scols = slice(st * 128, st * 128 + 128)
                    y_sb = rope.tile([128, 512], F32, tag="dst", name="y_sb")
                    nc.vector.tensor_copy(y_sb[:], ops[st][:])
                    nc.sync.dma_start(y_out[scols, ecols], y_sb[:])
    nc.compile()
    return nc


def _get_nc():
    if "nc" not in _cache:
        _cache["nc"] = _build_nc()
    return _cache["nc"]


def _head_perm():
    p = []
    for h in range(H):
        base = h * HD
        p += [base + 2 * j for j in range(HD // 2)]
        p += [base + 2 * j + 1 for j in range(HD // 2)]
    return np.array(p)


def _pack_thin(wT):
    # [2048(k), 2048(d)] -> [H, 128(p), KC, 128(d)] with chunk [h] contiguous
    return np.ascontiguousarray(
        wT.reshape(KC, 128, H, 128).transpose(2, 1, 0, 3)
    )


def _pack_fat(wT):
    # [2048(k), 2048(e)] -> [4(kch), 4(eb), 128(p), 4(kcq), 512(e)]
    return np.ascontiguousarray(
        wT.reshape(4, 4, 128, 4, 512).transpose(0, 3, 2, 1, 4)
    )


def _pack_x(xb):
    # [rows, 2048] -> [128(p), KC, rows]
    return np.ascontiguousarray(xb.T.reshape(KC, 128, -1).transpose(1, 0, 2))


def kernel(**inputs):
    from concourse.bass_utils import run_bass_kernel_spmd

    trace = bool(inputs.pop("_trace", False))
    x = np.asarray(inputs["x"], np.float32)
    freqs_cos = np.asarray(inputs["freqs_cos"], np.float32)
    freqs_sin = np.asarray(inputs["freqs_sin"], np.float32)
    mask = np.asarray(inputs["mask"], np.float32)
    wq = np.asarray(inputs["wq"], np.float32)
    bq = np.asarray(inputs["bq"], np.float32)
    wk = np.asarray(inputs["wk"], np.float32)
    bk = np.asarray(inputs["bk"], np.float32)
    wv = np.asarray(inputs["wv"], np.float32)
    bv = np.asarray(inputs["bv"], np.float32)
    wo = np.asarray(inputs["wo"], np.float32)
    bo = np.asarray(inputs["bo"], np.float32)
    start_pos = int(np.asarray(inputs.get("start_pos", 0)))

    perm = _head_perm()
    wq_pre = _pack_thin(np.ascontiguousarray(wq[perm].T))
    wk_pre = _pack_thin(np.ascontiguousarray(wk[perm].T))
    wv_pre = _pack_fat(np.ascontiguousarray(wv.T))
    wo_pre = _pack_fat(np.ascontiguousarray(wo.T))
    bq_p = np.ascontiguousarray(bq[perm].reshape(KC, 128, 1).transpose(1, 0, 2))
    bk_p = np.ascontiguousarray(bk[perm].reshape(KC, 128, 1).transpose(1, 0, 2))
    bv128 = np.ascontiguousarray(np.broadcast_to(bv[None, :], (128, DIM)))

    # rotary tables, rows [start_pos, start_pos+S)
    cosT = freqs_cos[start_pos:start_pos + S].T.astype(np.float32)  # [64, S]
    sinT = freqs_sin[start_pos:start_pos + S].T.astype(np.float32)
    csk2 = np.ascontiguousarray(np.vstack([cosT, cosT]))
    ssk2 = np.ascontiguousarray(np.vstack([-sinT, sinT]))

    m2 = mask[0, 0]  # [S(q), S(k)] additive
    ones128 = np.ones((128, 128), np.float32)
    ident128 = np.eye(128, dtype=np.float32)

    common = {
        "wq_pre": wq_pre, "wk_pre": wk_pre, "wv_pre": wv_pre, "wo_pre": wo_pre,
        "bq_p": bq_p, "bk_p": bk_p, "bv128": bv128,
        "ones128": ones128, "ident128": ident128,
    }
    in_maps = []
    for c in range(N_CORES):
        b, half = c // 2, c % 2
        q0 = half * NQ
        # rotated row order: own q-half first, then the complement
        rot = np.concatenate([np.arange(q0, q0 + NQ),
                              np.arange(0, q0),
                              np.arange(q0 + NQ, S)])
        amask = np.ascontiguousarray(
            m2[q0:q0 + NQ, :][:, rot].T.reshape(NKT, 128, NQ).transpose(1, 0, 2)
        )
        in_maps.append({
            **common,
            "x_pre": _pack_x(x[b][rot]),
            "csk2": np.ascontiguousarray(csk2[:, rot]),
            "ssk2": np.ascontiguousarray(ssk2[:, rot]),
            "amask": amask,
        })

    nc = _get_nc()
    kwargs = {}
    if trace:
        kwargs = {"trace": True, "trace_cores": list(range(N_CORES))}
    res = run_bass_kernel_spmd(nc, in_maps, core_ids=list(range(N_CORES)), **kwargs)
    _cache["last_result"] = res

    out = np.empty((B, S, DIM), np.float32)
    for c in range(N_CORES):
        b, half = c // 2, c % 2
        out[b, half * NQ:half * NQ + NQ] = res.results[c]["y"] + bo[None, :]
    return out



# revision 2
# speedup vs baseline: 1.1566x; 1.1566x over previous
"""Multi-head causal attention (LLaMA RoPE) on 8 Trainium2 cores, v6.

Sharding: core c -> (batch b = c//2, head-half hh = c%2, i.e. heads
[8*hh, 8*hh+8)). Each core projects Q/K/V for its 8 heads over all 1024
rows (no duplicated projection work), runs causal attention for those
heads over the full sequence, then pairs (2b, 2b+1) exchange attention
outputs with staggered pair-wise AllGathers so each core can apply the
full output projection for the 512 rows it owns (core 2b: rows 0-511,
core 2b+1: rows 512-1023). Per-core program is identical; per-core
behavior comes from input data + partition_id (drives the dynamic DMA
offsets selecting "my rows" / "partner slot").

Numerics: all matmul operands bf16 (PE full rate at any tile size),
accumulation fp32 in PSUM, biases/normalization fp32. Softmax skips
max-subtraction (logits are O(5)); denominator comes from a ones-matmul
accumulated alongside PV. Causality: k-tiles strictly above the
diagonal are skipped; diagonal 128x128 blocks are zeroed after exp with
a multiplicative 0/1 mask on the DVE (derived from the mask input).

Attention processes full q-strips [kt*128, 1024) per k-tile with
2-bank (4KB/partition) PSUM tiles: one exp per k-tile keeps the Act
engine under the PE's work, and a depth-3 software pipeline (PV and
denominator matmuls for k-tile t issue under ST of k-tile t+3) covers
both the exp latency and the per-head normalize drain. The output
projection's weights and attention-exchange chunks stream during
attention into buffers whose previous tenants (x, projection weights)
are dead, so the O-projection starts immediately at attention end with
all but the last gather's chunks resident.
"""

import math
import sys

import numpy as np

sys.path.insert(0, "/opt/trn_rl_repo")

from ml_dtypes import bfloat16

B, S, DIM, H = 4, 1024, 2048, 16
HD = DIM // H  # 128
HPC = 8  # heads per core
KC = DIM // 128  # 16 contraction chunks
NKT = S // 128  # 8 k tiles
NQ = S // 2  # 512 rows owned per core
SCALE = 1.0 / math.sqrt(HD)
N_CORES = 8
GROUPS = [[0, 1], [2, 3], [4, 5], [6, 7]]

_cache = {}


def _build_nc():
    import concourse.bass as bass
    import concourse.mybir as mybir
    import concourse.tile as tile
    from concourse import bacc

    BF16 = mybir.dt.bfloat16
    F32 = mybir.dt.float32
    ds = bass.ds
    Act = mybir.ActivationFunctionType

    nc = bacc.Bacc("TRN2", target_bir_lowering=False, debug=False,
                   num_devices=N_CORES)

    x_in = nc.dram_tensor("x_pre", [128, KC, S], BF16, kind="ExternalInput")
    wq_in = nc.dram_tensor("wq_pre", [HPC, 128, KC, 128], BF16, kind="ExternalInput")
    wk_in = nc.dram_tensor("wk_pre", [HPC, 128, KC, 128], BF16, kind="ExternalInput")
    wv_in = nc.dram_tensor("wv_pre", [128, KC, 2, 512], BF16, kind="ExternalInput")
    wo_in = nc.dram_tensor("wo_pre", [128, KC, 4, 512], BF16, kind="ExternalInput")
    bq_in = nc.dram_tensor("bq_p", [128, HPC, 1], F32, kind="ExternalInput")
    bk_in = nc.dram_tensor("bk_p", [128, HPC, 1], F32, kind="ExternalInput")
    bv_in = nc.dram_tensor("bv_p", [128, 2, 512], F32, kind="ExternalInput")
    csk_in = nc.dram_tensor("csk2", [128, S], BF16, kind="ExternalInput")
    ssk_in = nc.dram_tensor("ssk2", [128, S], BF16, kind="ExternalInput")
    tri_in = nc.dram_tensor("tri01", [128, 128], BF16, kind="ExternalInput")
    ones_in = nc.dram_tensor("ones128", [128, 128], BF16, kind="ExternalInput")
    y_out = nc.dram_tensor("y", [NQ, DIM], F32, kind="ExternalOutput")

    with tile.TileContext(nc) as tc:
        with (
            tc.tile_pool(name="consts", bufs=1) as consts,
            tc.tile_pool(name="xpool", bufs=1) as xpool,
            tc.tile_pool(name="qkv", bufs=1) as qkv,
            tc.tile_pool(name="wqk", bufs=4) as wqk,
            tc.tile_pool(name="combp", bufs=4) as combp,
            tc.tile_pool(name="wbig", bufs=2) as wbig,
            tc.tile_pool(name="rope", bufs=3) as rope,
            tc.tile_pool(name="ptp", bufs=5) as ptp,
            tc.tile_pool(name="dram", bufs=1, space="DRAM") as dram,
            tc.tile_pool(name="psB", bufs=2, space="PSUM") as psB,
            tc.tile_pool(name="psL1", bufs=1, space="PSUM") as psL1,
            tc.tile_pool(name="psO1", bufs=1, space="PSUM") as psO1,
        ):
            # ---- DMAs in consumption order: first Q/K weights + x ----
            wq_sbs = {}
            wk_sbs = {}

            def fetch_w(which, h, nsplit=1):
                t = wqk.tile([128, KC, 128], BF16, tag="w",
                             name=f"w{which}_sb")
                src = (wq_in if which == "q" else wk_in)[h]
                step = KC // nsplit
                for j in range(0, KC, step):
                    nc.sync.dma_start(t[:, j:j + step, :], src[:, j:j + step, :])
                (wq_sbs if which == "q" else wk_sbs)[h] = t

            # x streams on the Act engine's DMA queue, weights on sync's,
            # so the two overlap from t=0. First tiles split fine so the
            # first matmul's semaphore wait covers minimal data.
            x_sb = xpool.tile([128, KC, S], BF16, tag="x", name="x_sb")
            fetch_w("q", 0, nsplit=4)
            nc.scalar.dma_start(x_sb[:, 0:1, :], x_in[:, 0:1, :])
            nc.scalar.dma_start(x_sb[:, 1:2, :], x_in[:, 1:2, :])
            fetch_w("k", 0, nsplit=2)
            nc.scalar.dma_start(x_sb[:, 2:4, :], x_in[:, 2:4, :])
            bq_sb = consts.tile([128, HPC, 1], F32, tag="bq")
            bk_sb = consts.tile([128, HPC, 1], F32, tag="bk")
            nc.sync.dma_start(bq_sb[:], bq_in[:])
            nc.sync.dma_start(bk_sb[:], bk_in[:])
            fetch_w("q", 1)
            fetch_w("k", 1)
            for g in range(2, 8):
                nc.scalar.dma_start(x_sb[:, g * 2:g * 2 + 2, :],
                                    x_in[:, g * 2:g * 2 + 2, :])
            csk_sb = consts.tile([128, S], BF16, tag="cs", name="csk_sb")
            ssk_sb = consts.tile([128, S], BF16, tag="ss", name="ssk_sb")
            nc.sync.dma_start(csk_sb[:], csk_in[:])
            nc.sync.dma_start(ssk_sb[:], ssk_in[:])
            ones_sb = consts.tile([128, 128], BF16)
            nc.sync.dma_start(ones_sb[:], ones_in[:])
            tri_sb = consts.tile([128, 128], BF16)
            nc.sync.dma_start(tri_sb[:], tri_in[:])
            bv_sb = consts.tile([128, 2, 512], F32, tag="bv")
            nc.sync.dma_start(bv_sb[:], bv_in[:])

            q_sb = qkv.tile([128, HPC, S], BF16, tag="q", name="q_sb")
            k_sb = qkv.tile([128, HPC, S], BF16, tag="k", name="k_sb")
            v_sb = qkv.tile([128, NKT, 2, 512], BF16, tag="v", name="v_sb")
            ot_sb = qkv.tile([128, HPC, S], BF16, tag="ot", name="ot_sb")

            def rope_block(pm, b_ap, dst_ap):
                """dst = rope(pm + bias); pm [128, S] psum."""
                tmp_t = rope.tile([128, S], BF16, tag="tmp", name="tmp_t")
                nc.scalar.activation(tmp_t[:], pm, Act.Identity, bias=b_ap)
                tswap_t = rope.tile([128, S], BF16, tag="tswap", name="tswap_t")
                nc.sync.dma_start(tswap_t[0:64, :], tmp_t[64:128, :])
                nc.sync.dma_start(tswap_t[64:128, :], tmp_t[0:64, :])
                nc.vector.tensor_mul(tmp_t[:], tmp_t[:], csk_sb[:])
                nc.vector.tensor_mul(tswap_t[:], tswap_t[:], ssk_sb[:])
                nc.vector.tensor_add(dst_ap, tmp_t[:], tswap_t[:])

            # ---- Q/K projections + RoPE (weights double-prefetched) ----
            HALVES = [slice(0, 512), slice(512, 1024)]
            for h in range(HPC):
                wq_sb = wq_sbs[h]
                pm = psB.tile([128, S], F32, tag="big", name="pmq")
                for kc in range(KC):
                    for hs in HALVES:
                        nc.tensor.matmul(pm[:, hs], wq_sb[:, kc, :],
                                         x_sb[:, kc, hs],
                                         start=(kc == 0), stop=(kc == KC - 1))
                rope_block(pm[:], bq_sb[:, h, :], q_sb[:, h, :])
                if h + 2 < HPC:
                    fetch_w("q", h + 2)
                wk_sb = wk_sbs[h]
                pm = psB.tile([128, S], F32, tag="big", name="pmk")
                for kc in range(KC):
                    for hs in HALVES:
                        nc.tensor.matmul(pm[:, hs], wk_sb[:, kc, :],
                                         x_sb[:, kc, hs],
                                         start=(kc == 0), stop=(kc == KC - 1))
                rope_block(pm[:], bk_sb[:, h, :], k_sb[:, h, :])
                if h + 2 < HPC:
                    fetch_w("k", h + 2)

            # ---- V projection (transposed: rows on partitions) ----
            for eb in range(2):
                wv_sb = wbig.tile([128, KC, 512], BF16, tag="wv", name="wv_sb")
                nc.sync.dma_start(wv_sb[:], wv_in[:, :, eb, :])
                for rc in range(NKT):
                    rcols = slice(rc * 128, rc * 128 + 128)
                    pm = psB.tile([128, S], F32, tag="big", name="pmv")
                    for kc in range(KC):
                        nc.tensor.matmul(pm[:, 0:512], x_sb[:, kc, rcols],
                                         wv_sb[:, kc, :],
                                         start=(kc == 0), stop=(kc == KC - 1))
                    nc.vector.tensor_add(v_sb[:, rc, eb, :], pm[:, 0:512],
                                         bv_sb[:, eb, :])

            # ---- pair-exchange plumbing ----
            pid = nc.sync.partition_id()
            myrh = pid % 2
            prh = (pid + 1) % 2
            # staggered gathers: heads 0-3, 4-5, 6, 7 (tail kept tiny so
            # the final gather lands before the O-projection needs it)
            AG_HEADS = [(0, 4), (4, 2), (6, 1), (7, 1)]
            cc_in = [dram.tile([128, nh, 512], BF16, name=f"cc_in{g}")
                     for g, (h0, nh) in enumerate(AG_HEADS)]
            cc_out = [dram.tile([2, 128, nh, 512], BF16, name=f"cc_out{g}")
                      for g, (h0, nh) in enumerate(AG_HEADS)]
            # comb quarters: O-projection d_in chunks, local-first order
            # (0-7 = my heads for my rows, 8-15 = partner heads; wo_pre is
            # packed per-core with the matching chunk order). Quarters so
            # each lands as soon as its heads/gather finish.
            combq = [combp.tile([128, 4, 512], BF16, tag="cq",
                                name=f"combq{j}") for j in range(4)]
            wo23 = []

            # ---- attention: full q-strips, depth-3 pipeline ----
            def vtile(h, kt):
                return v_sb[:, kt, h // 4, (h % 4) * 128:(h % 4) * 128 + 128]

            for h in range(HPC):
                l_ps = psL1.tile([128, S], F32, tag="l", name="l_ps")
                o_ps = psO1.tile([128, S], F32, tag="o", name="o_ps")
                pts = []

                def segs(kt):
                    lo = kt * 128
                    out = []
                    if lo < 512:
                        out.append(slice(lo, 512))
                        out.append(slice(512, 1024))
                    else:
                        out.append(slice(lo, 1024))
                    return out

                def emit_lo(kt):
                    for sg in segs(kt):
                        nc.tensor.matmul(l_ps[:, sg], ones_sb[:],
                                         pts[kt][:, sg],
                                         start=(kt == 0), stop=(kt == NKT - 1))
                    for sg in segs(kt):
                        nc.tensor.matmul(o_ps[:, sg], vtile(h, kt),
                                         pts[kt][:, sg],
                                         start=(kt == 0), stop=(kt == NKT - 1))

                for kt in range(NKT):
                    lo = kt * 128
                    st = psB.tile([128, S], F32, tag="big", name="st_ps")
                    for sg in segs(kt):
                        nc.tensor.matmul(
                            st[:, sg], k_sb[:, h, kt * 128:kt * 128 + 128],
                            q_sb[:, h, sg], start=True, stop=True)
                    pt = ptp.tile([128, S], BF16, tag="pt", name="pt")
                    pts.append(pt)
                    nc.scalar.activation(pt[:, lo:1024], st[:, lo:1024],
                                         Act.Exp, scale=SCALE)
                    # diagonal block: zero out q < k after exp
                    nc.vector.tensor_mul(pt[:, lo:lo + 128],
                                         pt[:, lo:lo + 128], tri_sb[:])
                    if kt >= 3:
                        emit_lo(kt - 3)
                for kt in range(NKT - 3, NKT):
                    emit_lo(kt)
                rl = consts.tile([128, S], F32, tag="rl", name="rl")
                nc.vector.reciprocal_approx_fast(rl[:], l_ps[:])
                nc.vector.tensor_mul(ot_sb[:, h, :], o_ps[:], rl[:])
                # my-rows slice of this head -> comb chunk h (static dst)
                nc.sync.dma_start(combq[h // 4][:, h % 4, :],
                                  ot_sb[:, h, ds(myrh * 512, 512)])
                if h in (3, 5, 6, 7):
                    g = {3: 0, 5: 1, 6: 2, 7: 3}[h]
                    h0, nh = AG_HEADS[g]
                    nc.sync.dma_start(
                        cc_in[g][:],
                        ot_sb[:, h0:h0 + nh, ds(prh * 512, 512)])
                    nc.gpsimd.collective_compute(
                        "AllGather",
                        mybir.AluOpType.bypass,
                        replica_groups=GROUPS,
                        ins=[cc_in[g][:].opt()],
                        outs=[cc_out[g][:].opt()],
                    )
                    # partner slot -> comb chunks 8+ (static dst)
                    coff = HPC + sum(n for _, n in AG_HEADS[:g])
                    nc.sync.dma_start(
                        combq[coff // 4][:, coff % 4:coff % 4 + nh, :],
                        cc_out[g][ds(prh, 1)][0])
                # stream wo during attention: ob 0-1 into the dead x_sb
                # buffer, ob 2-3 into the wv pool
                if h == 3:
                    wo01_sb = xpool.tile([128, KC, 2, 512], BF16, tag="x",
                                         name="wo01_sb")
                    nc.sync.dma_start(wo01_sb[:], wo_in[:, :, 0:2, :])
                elif h in (5, 6):
                    t = wbig.tile([128, KC, 512], BF16, tag="wv",
                                  name="wo_sb")
                    nc.sync.dma_start(t[:], wo_in[:, :, h - 3, :])
                    wo23.append(t)

            # ---- output projection for my 512 rows ----
            # gathers' latency hides under the local-chunk matmuls;
            # alternate PSUM pools between ob blocks so block N+1 never
            # waits on block N's drains.
            for ob in range(4):
                if ob % 2 == 0:
                    pms2 = [psB.tile([128, S], F32, tag="big", name="pmo01"),
                            psB.tile([128, S], F32, tag="big", name="pmo23")]
                else:
                    pms2 = [psL1.tile([128, S], F32, tag="l", name="pmo01"),
                            psO1.tile([128, S], F32, tag="o", name="pmo23")]
                for i in range(KC):
                    for qc in range(4):
                        qsl = slice(qc * 128, qc * 128 + 128)
                        osl = slice((qc % 2) * 512, (qc % 2) * 512 + 512)
                        nc.tensor.matmul(pms2[qc // 2][:, osl],
                                         combq[i // 4][:, i % 4, qsl],
                                         (wo01_sb[:, i, ob, :] if ob < 2
                                          else wo23[ob - 2][:, i, :]),
                                         start=(i == 0), stop=(i == KC - 1))
                for qc in range(4):
                    qsl = slice(qc * 128, qc * 128 + 128)
                    osl = slice((qc % 2) * 512, (qc % 2) * 512 + 512)
                    y_sb = rope.tile([128, 512], F32, tag="tswap", name="y_sb")
                    if qc % 2 == 0:
                        nc.vector.tensor_copy(y_sb[:], pms2[qc // 2][:, osl])
                    else:
                        nc.scalar.activation(y_sb[:], pms2[qc // 2][:, osl],
                                             Act.Identity)
                    nc.sync.dma_start(y_out[qsl, ob * 512:ob * 512 + 512],
                                      y_sb[:])
    nc.compile()
    return nc


def _get_nc():
    if "nc" not in _cache:
        _cache["nc"] = _build_nc()
    return _cache["nc"]


def _evenodd(a):
    # permute within-head dim: even indices first, then odd (axis 0)
    return np.concatenate([a[0::2], a[1::2]], axis=0)


def kernel(**inputs):
    from concourse.bass_utils import run_bass_kernel_spmd

    trace = bool(inputs.pop("_trace", False))
    x = np.asarray(inputs["x"], np.float32)
    freqs_cos = np.asarray(inputs["freqs_cos"], np.float32)
    freqs_sin = np.asarray(inputs["freqs_sin"], np.float32)
    mask = np.asarray(inputs["mask"], np.float32)
    wq = np.asarray(inputs["wq"], np.float32)
    bq = np.asarray(inputs["bq"], np.float32)
    wk = np.asarray(inputs["wk"], np.float32)
    bk = np.asarray(inputs["bk"], np.float32)
    wv = np.asarray(inputs["wv"], np.float32)
    bv = np.asarray(inputs["bv"], np.float32)
    wo = np.asarray(inputs["wo"], np.float32)
    bo = np.asarray(inputs["bo"], np.float32)

    cosT = freqs_cos.T
    sinT = freqs_sin.T
    csk2 = np.vstack([cosT, cosT]).astype(bfloat16)
    ssk2 = np.vstack([-sinT, sinT]).astype(bfloat16)

    m2 = mask[0, 0]  # [S(q), S(k)] additive
    # multiplicative 0/1 within-block causal mask, [k, q] layout
    tri01 = (m2[:128, :128].T == 0.0).astype(np.float32).astype(bfloat16)
    ones128 = np.ones((128, 128), np.float32).astype(bfloat16)

    def pack_thin(w_half):
        out = np.empty((HPC, 128, KC, 128), bfloat16)
        for h in range(HPC):
            rows = _evenodd(w_half[h * 128:(h + 1) * 128])  # [128, 2048]
            out[h] = rows.T.reshape(KC, 128, 128).transpose(1, 0, 2).astype(bfloat16)
        return out

    halves = []
    for hh in range(2):
        sl = slice(hh * 1024, hh * 1024 + 1024)
        wq_pre = pack_thin(wq[sl])
        wk_pre = pack_thin(wk[sl])
        bq_p = np.empty((128, HPC, 1), np.float32)
        bk_p = np.empty((128, HPC, 1), np.float32)
        for h in range(HPC):
            bq_p[:, h, 0] = _evenodd(bq[hh * 1024 + h * 128:hh * 1024 + (h + 1) * 128])
            bk_p[:, h, 0] = _evenodd(bk[hh * 1024 + h * 128:hh * 1024 + (h + 1) * 128])
        wv_pre = np.ascontiguousarray(
            wv[sl].T.reshape(KC, 128, 2, 512).transpose(1, 0, 2, 3)
        ).astype(bfloat16)
        # wo d_in chunks rolled so local head chunks come first (matches
        # comb's local-first layout in the kernel)
        order = list(range(hh * 8, hh * 8 + 8)) + \
            list(range((1 - hh) * 8, (1 - hh) * 8 + 8))
        woT = wo.T.reshape(KC, 128, 4, 512)
        wo_pre = np.ascontiguousarray(
            woT[order].transpose(1, 0, 2, 3)
        ).astype(bfloat16)
        halves.append((wq_pre, wk_pre, bq_p, bk_p, wv_pre, wo_pre))

    common = {
        "csk2": csk2, "ssk2": ssk2,
        "tri01": tri01, "ones128": ones128,
    }
    in_maps = []
    for c in range(N_CORES):
        b, hh = c // 2, c % 2
        wq_pre, wk_pre, bq_p, bk_p, wv_pre, wo_pre = halves[hh]
        x_pre = np.ascontiguousarray(
            x[b].T.reshape(KC, 128, S).transpose(1, 0, 2)
        ).astype(bfloat16)
        bv_p = np.ascontiguousarray(
            np.broadcast_to(bv[hh * 1024:hh * 1024 + 1024].reshape(1, 2, 512),
                            (128, 2, 512))
        )
        in_maps.append({
            **common,
            "x_pre": x_pre,
            "wq_pre": wq_pre, "wk_pre": wk_pre,
            "bq_p": bq_p, "bk_p": bk_p,
            "wv_pre": wv_pre, "bv_p": bv_p, "wo_pre": wo_pre,
        })

    nc = _get_nc()
    kwargs = {}
    if trace:
        kwargs = {"trace": True, "trace_cores": list(range(N_CORES))}
    res = run_bass_kernel_spmd(nc, in_maps, core_ids=list(range(N_CORES)), **kwargs)
    _cache["last_result"] = res

    out = np.empty((B, S, DIM), np.float32)
    for c in range(N_CORES):
        b, hh = c // 2, c % 2
        out[b, hh * NQ:hh * NQ + NQ] = res.results[c]["y"] + bo[None, :]
    return out
